# revision 16
# baseline (speedup 1.0000x reference)
"""Multi-head attention (16 heads, d_model=1024, bs=2, qlen=2048) on 8 trn2 cores.

Sharding: core c -> batch b = c//4, head-group r = c%4 (heads 4r..4r+3, i.e.
dims 256r..256r+256 of the head axis).  Each core projects q/k/v only for its
own 4 heads (Megatron column split), runs scores/softmax/AV for those heads,
then an AllGather of the per-core context slice within each batch group of 4
cores provides the full 1024-dim context for the row-split output projection
(each core computes its own 256 output columns; no reduction needed).  The
AllGather + output projection are split per 512-token tile and overlap the
attention of later tiles.

Numerics: bf16 matmul operands, fp32 PSUM accumulation, softmax in fp32 on the
scalar engine.  Scores are computed transposed (k on partitions) so the mask
is a per-partition multiply and the softmax denominator comes from an M=1
ones-matmul; the reciprocal is computed on a single partition row and
broadcast across partitions with a K=1 ones-matmul (elementwise normalize, no
cross-partition vector ops).  1/sqrt(d) and q_b are folded into q_w/q_b on the
host; v_b is deferred past the softmax (rows of P/sum sum to 1) and added to
the normalized context.
"""

import functools
import os
import sys

import numpy as np

for _p in ("/opt/trn_rl_repo", "/root/.axon_site/_ro/trn_rl_repo"):
    if os.path.isdir(_p) and _p not in sys.path:
        sys.path.append(_p)

import ml_dtypes

from concourse import bacc, bass, mybir, tile
from concourse.bass_utils import run_bass_kernel_spmd

BF16 = ml_dtypes.bfloat16
FP32 = mybir.dt.float32
BF16_DT = mybir.dt.bfloat16

N_CORES = 8
BS = 2
L = 2048  # sequence length
D = 1024  # model dim
DH = 64  # head dim
OWN = 256  # head dims per core (4 heads)
KC_D = 8  # 1024 / 128 contraction chunks for projections
NT = 4  # 2048 / 512 token tiles
KT = 16  # 2048 / 128 key-token chunks
ACT_GRP = 2  # k-chunks per exp() activation op
WARMUP_MM = 28  # dummy matmuls to warm the PE clock gate during input DMA

LAST_EXEC_NS = None
LAST_RESULTS = None


def _build_nc(apply_mask: bool):
    nc = bacc.Bacc(None, num_devices=N_CORES)

    xT = nc.dram_tensor("xT", [D, L], BF16_DT, kind="ExternalInput")
    wq = nc.dram_tensor("wq", [D, OWN], BF16_DT, kind="ExternalInput")
    wk = nc.dram_tensor("wk", [D, OWN], BF16_DT, kind="ExternalInput")
    wv = nc.dram_tensor("wv", [D, OWN], BF16_DT, kind="ExternalInput")
    wo = nc.dram_tensor("wo", [D, OWN], BF16_DT, kind="ExternalInput")
    qb2 = nc.dram_tensor("qb2", [128, 2], FP32, kind="ExternalInput")
    kb2 = nc.dram_tensor("kb2", [128, 2], FP32, kind="ExternalInput")
    vb2 = nc.dram_tensor("vb2", [128, 2], FP32, kind="ExternalInput")
    ob2 = nc.dram_tensor("ob2", [128, 2], FP32, kind="ExternalInput")
    mask01 = nc.dram_tensor("mask01", [128, KT], FP32, kind="ExternalInput")
    outT = nc.dram_tensor("outT", [OWN, L], FP32, kind="ExternalOutput")

    Exp = mybir.ActivationFunctionType.Exp

    with tile.TileContext(nc) as tc:
        with (
            tc.tile_pool(name="const", bufs=1) as const,
            tc.tile_pool(name="work", bufs=2) as work,
            tc.tile_pool(name="ps", bufs=1, space="PSUM") as ps,
            tc.tile_pool(name="dram", bufs=1, space="DRAM") as dram,
        ):
            # ---- stage inputs into SBUF ----
            x_sb = []
            for i in range(KC_D):
                t = const.tile([128, L], BF16_DT, tag=f"x{i}", name=f"x_sb{i}")
                nc.sync.dma_start(t, xT[i * 128 : (i + 1) * 128, :])
                x_sb.append(t)

            def load_w(dram_t, nm):
                tiles = []
                for i in range(KC_D):
                    t = const.tile([128, OWN], BF16_DT, tag=f"{nm}{i}", name=f"{nm}_sb{i}")
                    nc.sync.dma_start(t, dram_t[i * 128 : (i + 1) * 128, :])
                    tiles.append(t)
                return tiles

            wk_sb = load_w(wk, "wk")
            wq_sb = load_w(wq, "wq")
            wv_sb = load_w(wv, "wv")
            wo_sb = load_w(wo, "wo")

            def load_small(dram_t, nm, cols):
                t = const.tile([128, cols], FP32, tag=nm, name=f"{nm}_sb")
                nc.sync.dma_start(t, dram_t[:, :])
                return t

            qb_sb = load_small(qb2, "qb", 2)
            kb_sb = load_small(kb2, "kb", 2)
            vb_sb = load_small(vb2, "vb", 2)
            ob_sb = load_small(ob2, "ob", 2)
            mask_sb = load_small(mask01, "mask", KT) if apply_mask else None

            ones_sb = const.tile([128, DH], BF16_DT, tag="ones", name="ones_sb")
            nc.vector.memset(ones_sb, 1.0)
            onesf_sb = const.tile([128, DH], FP32, tag="onesf", name="onesf_sb")
            nc.vector.memset(onesf_sb, 1.0)

            # Warm the PE clock gate while inputs stream in: dependency-free
            # matmuls on the ones tile keep the HAM busy-window active.
            wsc = const.tile([128, 512], BF16_DT, tag="wsc", name="wsc")
            nc.vector.memset(wsc, 0.0)
            wps = ps.tile([128, 512], FP32, tag="sc", bufs=2, name="wps")
            for _ in range(WARMUP_MM):
                nc.tensor.matmul(wps[0:64, :], lhsT=ones_sb[:, 0:64], rhs=wsc)

            # ---- k/q projections (transposed: own-dim on partitions) ----
            qT_sb = [
                const.tile([128, L], BF16_DT, tag=f"qT{p}", name=f"qT_sb{p}")
                for p in range(2)
            ]
            kT_sb = [
                const.tile([128, L], BF16_DT, tag=f"kT{p}", name=f"kT_sb{p}")
                for p in range(2)
            ]
            for w_sb, b_sb, dst in ((wk_sb, kb_sb, kT_sb), (wq_sb, qb_sb, qT_sb)):
                for m in range(2):
                    for n in range(NT):
                        pp = ps.tile(
                            [128, 512], FP32, tag="sc", bufs=2, name="proj_ps",
                        )
                        for kc in range(KC_D):
                            nc.tensor.matmul(
                                pp,
                                lhsT=w_sb[kc][:, m * 128 : (m + 1) * 128],
                                rhs=x_sb[kc][:, n * 512 : (n + 1) * 512],
                                start=(kc == 0),
                                stop=(kc == KC_D - 1),
                            )
                        nc.vector.tensor_scalar_add(
                            dst[m][:, n * 512 : (n + 1) * 512], pp, b_sb[:, m : m + 1]
                        )

            # ---- v projection (untransposed: tokens on partitions; no bias) ----
            # Stored interleaved with ones columns so a single M=128 matmul
            # per head produces both context rows and softmax-denominator
            # rows: per pair block of 256 cols: [v_h0 | ones | ones | v_h1].
            v_sb = [
                const.tile([128, 512], BF16_DT, tag=f"v{t}", name=f"v_sb{t}")
                for t in range(KT)
            ]
            for t in range(KT):
                nc.vector.memset(v_sb[t][:, 64:192], 1.0)
                nc.vector.memset(v_sb[t][:, 320:448], 1.0)
                pv = ps.tile([128, OWN], FP32, tag="sc", bufs=2, name="v_ps")
                for kc in range(KC_D):
                    nc.tensor.matmul(
                        pv,
                        lhsT=x_sb[kc][:, t * 128 : (t + 1) * 128],
                        rhs=wv_sb[kc],
                        start=(kc == 0),
                        stop=(kc == KC_D - 1),
                    )
                nc.vector.tensor_copy(v_sb[t][:, 0:64], pv[:, 0:64])
                nc.vector.tensor_copy(v_sb[t][:, 192:256], pv[:, 64:128])
                nc.vector.tensor_copy(v_sb[t][:, 256:320], pv[:, 128:192])
                nc.vector.tensor_copy(v_sb[t][:, 448:512], pv[:, 192:256])

            # ---- attention + per-token-tile AllGather / output projection ----
            ctx_sb = [
                const.tile([128, L], BF16_DT, tag=f"ctx{p}", name=f"ctx_sb{p}")
                for p in range(2)
            ]
            n_grp = KT // ACT_GRP
            def oproj(qt):
                # output projection for token tile qt (own 256 columns);
                # emitted one tile behind attention so the tensor engine never
                # queues behind the AllGather latency
                qsl = slice(qt * 512, (qt + 1) * 512)
                cf = []
                for kc in range(KC_D):
                    t = work.tile([128, 512], BF16_DT, tag=f"cf{kc}", name=f"cf{kc}")
                    nc.sync.dma_start(t, ag_out[qt][kc * 128 : (kc + 1) * 128, :])
                    cf.append(t)
                for m in range(2):
                    po = ps.tile([128, 512], FP32, tag="op", bufs=2, name="o_ps")
                    for kc in range(KC_D):
                        nc.tensor.matmul(
                            po,
                            lhsT=wo_sb[kc][:, m * 128 : (m + 1) * 128],
                            rhs=cf[kc],
                            start=(kc == 0),
                            stop=(kc == KC_D - 1),
                        )
                    osb = work.tile([128, 512], FP32, tag="osb", name="osb")
                    nc.vector.tensor_scalar_add(osb, po, ob_sb[:, m : m + 1])
                    nc.sync.dma_start(outT[m * 128 : (m + 1) * 128, qsl], osb)

            ag_out = []
            for qt in range(NT):
                qsl = slice(qt * 512, (qt + 1) * 512)
                for p in range(2):
                    # cs0: rows 0-63 ctx_h0, rows 64-127 denominators (x64)
                    # cs1: rows 0-63 denominators (x64), rows 64-127 ctx_h1
                    cs0 = ps.tile([128, 512], FP32, tag="cs0", bufs=1, name="cs0")
                    cs1 = ps.tile([128, 512], FP32, tag="cs1", bufs=1, name="cs1")
                    q0 = qT_sb[p][0:64, qsl]
                    q1 = qT_sb[p][64:128, qsl]

                    def av_group(g, pr0, pr1):
                        for j in range(ACT_GRP):
                            kc = g * ACT_GRP + j
                            sl = slice(j * 512, (j + 1) * 512)
                            st = kc == 0
                            sp = kc == KT - 1
                            nc.tensor.matmul(
                                cs0,
                                lhsT=v_sb[kc][:, p * 256 : p * 256 + 128],
                                rhs=pr0[:, sl],
                                start=st,
                                stop=sp,
                            )
                            nc.tensor.matmul(
                                cs1,
                                lhsT=v_sb[kc][:, p * 256 + 128 : p * 256 + 256],
                                rhs=pr1[:, sl],
                                start=st,
                                stop=sp,
                            )

                    prev = None
                    for g in range(n_grp):
                        s0 = ps.tile(
                            [128, 512 * ACT_GRP], FP32, tag="sc", bufs=2, name="s0"
                        )
                        s1 = ps.tile(
                            [128, 512 * ACT_GRP], FP32, tag="sc", bufs=2, name="s1"
                        )
                        for j in range(ACT_GRP):
                            kc = g * ACT_GRP + j
                            nc.tensor.matmul(
                                s0[:, j * 512 : (j + 1) * 512],
                                lhsT=kT_sb[p][0:64, kc * 128 : (kc + 1) * 128],
                                rhs=q0,
                            )
                            nc.tensor.matmul(
                                s1[:, j * 512 : (j + 1) * 512],
                                lhsT=kT_sb[p][64:128, kc * 128 : (kc + 1) * 128],
                                rhs=q1,
                            )
                        pr0 = work.tile(
                            [128, 512 * ACT_GRP], BF16_DT, tag="pr0", name="pr0"
                        )
                        pr1 = work.tile(
                            [128, 512 * ACT_GRP], BF16_DT, tag="pr1", name="pr1"
                        )
                        if apply_mask:
                            e0 = work.tile(
                                [128, 512 * ACT_GRP], FP32, tag="e0", name="e0"
                            )
                            e1 = work.tile(
                                [128, 512 * ACT_GRP], FP32, tag="e1", name="e1"
                            )
                            nc.scalar.activation(e0, s0, Exp)
                            nc.scalar.activation(e1, s1, Exp)
                            for j in range(ACT_GRP):
                                kc = g * ACT_GRP + j
                                sl = slice(j * 512, (j + 1) * 512)
                                nc.vector.tensor_scalar_mul(
                                    pr0[:, sl], e0[:, sl], mask_sb[:, kc : kc + 1]
                                )
                                nc.vector.tensor_scalar_mul(
                                    pr1[:, sl], e1[:, sl], mask_sb[:, kc : kc + 1]
                                )
                        else:
                            nc.scalar.activation(pr0, s0, Exp)
                            nc.scalar.activation(pr1, s1, Exp)
                        # AV runs one exp-group behind so the tensor engine
                        # never stalls waiting on the current group's exp
                        if prev is not None:
                            av_group(g - 1, *prev)
                        prev = (pr0, pr1)
                    av_group(n_grp - 1, *prev)

                    # normalize: copy one denominator row per head to SBUF,
                    # broadcast across partitions with K=1 ones-matmuls, take
                    # the reciprocal of the full base-0 bank (the custom-DVE
                    # reciprocal only supports partition base 0), multiply
                    srow = work.tile([128, 512], FP32, tag="srow", name="srow")
                    nc.vector.tensor_copy(srow[64:65, :], cs0[64:65, :])
                    nc.vector.tensor_copy(srow[0:1, :], cs1[0:1, :])
                    rb = ps.tile([128, 512], FP32, tag="op", bufs=2, name="rb")
                    nc.tensor.matmul(
                        rb[0:64, :], lhsT=onesf_sb[64:65, 0:64], rhs=srow[64:65, :]
                    )
                    nc.tensor.matmul(
                        rb[64:128, :], lhsT=onesf_sb[0:1, 0:64], rhs=srow[0:1, :]
                    )
                    recip = work.tile([128, 512], FP32, tag="recip", name="recip")
                    nc.vector.reciprocal_approx_fast(recip, rb)
                    tmp = work.tile([128, 512], FP32, tag="tmp", name="tmp")
                    nc.vector.tensor_mul(tmp[0:64, :], cs0[0:64, :], recip[0:64, :])
                    nc.vector.tensor_mul(
                        tmp[64:128, :], cs1[64:128, :], recip[64:128, :]
                    )
                    nc.vector.tensor_scalar_add(
                        ctx_sb[p][:, qsl], tmp, vb_sb[:, p : p + 1]
                    )

                # AllGather this token tile's context within the batch group
                ag_in = dram.tile([OWN, 512], BF16_DT, tag=f"agi{qt}", name=f"agi{qt}")
                ago = dram.tile([D, 512], BF16_DT, tag=f"ago{qt}", name=f"ago{qt}")
                ag_out.append(ago)
                nc.sync.dma_start(ag_in[0:128, :], ctx_sb[0][:, qsl])
                nc.sync.dma_start(ag_in[128:256, :], ctx_sb[1][:, qsl])
                nc.gpsimd.collective_compute(
                    "AllGather",
                    mybir.AluOpType.bypass,
                    replica_groups=[[0, 1, 2, 3], [4, 5, 6, 7]],
                    ins=[ag_in.opt()],
                    outs=[ago.opt()],
                )
                if qt > 0:
                    oproj(qt - 1)
            oproj(NT - 1)

    nc.finalize()
    return nc


@functools.lru_cache(maxsize=2)
def _built(apply_mask: bool):
    return _build_nc(apply_mask)


def kernel(input, mask, q_w, q_b, k_w, k_b, v_w, v_b, o_w, o_b):
    global LAST_EXEC_NS, LAST_RESULTS
    input = np.asarray(input, dtype=np.float32)
    mask = np.asarray(mask)
    apply_mask = not bool(np.all(mask != 0))
    nc = _built(apply_mask)

    qw = (np.asarray(q_w, np.float32) / 8.0).astype(BF16)
    kw = np.asarray(k_w, np.float32).astype(BF16)
    vw = np.asarray(v_w, np.float32).astype(BF16)
    ow = np.asarray(o_w, np.float32).astype(BF16)
    qb = np.asarray(q_b, np.float32) / 8.0
    kb = np.asarray(k_b, np.float32)
    vb = np.asarray(v_b, np.float32)
    ob = np.asarray(o_b, np.float32)

    in_maps = []
    for c in range(N_CORES):
        b, r = divmod(c, 4)
        own = slice(OWN * r, OWN * (r + 1))
        m01 = (mask[b] != 0).astype(np.float32)
        in_maps.append(
            {
                "xT": np.ascontiguousarray(input[b].T.astype(BF16)),
                "wq": np.ascontiguousarray(qw[own, :].T),
                "wk": np.ascontiguousarray(kw[own, :].T),
                "wv": np.ascontiguousarray(vw[own, :].T),
                "wo": np.ascontiguousarray(ow[own, :].T),
                "qb2": np.ascontiguousarray(qb[own].reshape(2, 128).T),
                "kb2": np.ascontiguousarray(kb[own].reshape(2, 128).T),
                "vb2": np.ascontiguousarray(vb[own].reshape(2, 128).T),
                "ob2": np.ascontiguousarray(ob[own].reshape(2, 128).T),
                "mask01": np.ascontiguousarray(m01.reshape(KT, 128).T),
            }
        )

    trace = os.environ.get("KERNEL_TRACE", "0") == "1"
    res = run_bass_kernel_spmd(
        nc,
        in_maps,
        core_ids=list(range(N_CORES)),
        trace=trace,
        trace_cores=list(range(N_CORES)) if trace else None,
        stitch_traces=False,
    )
    LAST_EXEC_NS = res.exec_time_ns
    LAST_RESULTS = res

    out = np.empty((BS, L, D), dtype=np.float32)
    for c in range(N_CORES):
        b, r = divmod(c, 4)
        out[b, :, OWN * r : OWN * (r + 1)] = res.results[c]["outT"].T
    return out


# revision 20
# speedup vs baseline: 1.1677x; 1.1677x over previous
"""Multi-head attention (16 heads, d_model=1024, bs=2, qlen=2048) on 8 trn2 cores.

Sharding: core c -> batch b = c//4, head-group r = c%4 (heads 4r..4r+3, i.e.
dims 256r..256r+256 of the head axis).  Each core projects q/k/v only for its
own 4 heads (Megatron column split), runs scores/softmax/AV for those heads,
then an AllGather of the per-core context slice within each batch group of 4
cores provides the full 1024-dim context for the row-split output projection
(each core computes its own 256 output columns; no reduction needed).  The
AllGather + output projection are split per 512-token tile and overlap the
attention of later tiles.

Numerics: bf16 matmul operands, fp32 PSUM accumulation, softmax in fp32 on the
scalar engine.  Scores are computed transposed (k on partitions) so the mask
is a per-partition multiply and the softmax denominator comes from an M=1
ones-matmul; the reciprocal is computed on a single partition row and
broadcast across partitions with a K=1 ones-matmul (elementwise normalize, no
cross-partition vector ops).  1/sqrt(d) and q_b are folded into q_w/q_b on the
host; v_b is deferred past the softmax (rows of P/sum sum to 1) and added to
the normalized context.
"""

import functools
import os
import sys

import numpy as np

for _p in ("/opt/trn_rl_repo", "/root/.axon_site/_ro/trn_rl_repo"):
    if os.path.isdir(_p) and _p not in sys.path:
        sys.path.append(_p)

import ml_dtypes

from concourse import bacc, bass, mybir, tile
from concourse.bass_utils import run_bass_kernel_spmd

BF16 = ml_dtypes.bfloat16
FP32 = mybir.dt.float32
BF16_DT = mybir.dt.bfloat16

N_CORES = 8
BS = 2
L = 2048  # sequence length
D = 1024  # model dim
DH = 64  # head dim
OWN = 256  # head dims per core (4 heads)
KC_D = 8  # 1024 / 128 contraction chunks for projections
NT = 4  # 2048 / 512 token tiles
KT = 16  # 2048 / 128 key-token chunks
ACT_GRP = 2  # k-chunks per exp() activation op
WARMUP_MM = 28  # dummy matmuls to warm the PE clock gate during input DMA

LAST_EXEC_NS = None
LAST_RESULTS = None


def _build_nc(apply_mask: bool):
    nc = bacc.Bacc(None, num_devices=N_CORES)

    xT = nc.dram_tensor("xT", [D, L], BF16_DT, kind="ExternalInput")
    wq = nc.dram_tensor("wq", [D, OWN], BF16_DT, kind="ExternalInput")
    wk = nc.dram_tensor("wk", [D, OWN], BF16_DT, kind="ExternalInput")
    wv = nc.dram_tensor("wv", [D, OWN], BF16_DT, kind="ExternalInput")
    wo = nc.dram_tensor("wo", [D, OWN], BF16_DT, kind="ExternalInput")
    qb2 = nc.dram_tensor("qb2", [128, 2], FP32, kind="ExternalInput")
    kb2 = nc.dram_tensor("kb2", [128, 2], FP32, kind="ExternalInput")
    vb2 = nc.dram_tensor("vb2", [128, 2], FP32, kind="ExternalInput")
    ob2 = nc.dram_tensor("ob2", [128, 2], FP32, kind="ExternalInput")
    mask01 = nc.dram_tensor("mask01", [128, KT], FP32, kind="ExternalInput")
    outT = nc.dram_tensor("outT", [OWN, L], FP32, kind="ExternalOutput")

    Exp = mybir.ActivationFunctionType.Exp

    with tile.TileContext(nc) as tc:
        with (
            tc.tile_pool(name="const", bufs=1) as const,
            tc.tile_pool(name="work", bufs=2) as work,
            tc.tile_pool(name="ps", bufs=1, space="PSUM") as ps,
            tc.tile_pool(name="dram", bufs=1, space="DRAM") as dram,
        ):
            # ---- stage inputs into SBUF ----
            x_sb = []
            for i in range(KC_D):
                t = const.tile([128, L], BF16_DT, tag=f"x{i}", name=f"x_sb{i}")
                nc.sync.dma_start(t, xT[i * 128 : (i + 1) * 128, :])
                x_sb.append(t)

            def load_w(dram_t, nm):
                tiles = []
                for i in range(KC_D):
                    t = const.tile([128, OWN], BF16_DT, tag=f"{nm}{i}", name=f"{nm}_sb{i}")
                    nc.sync.dma_start(t, dram_t[i * 128 : (i + 1) * 128, :])
                    tiles.append(t)
                return tiles

            wk_sb = load_w(wk, "wk")
            wq_sb = load_w(wq, "wq")
            wv_sb = load_w(wv, "wv")
            wo_sb = load_w(wo, "wo")

            def load_small(dram_t, nm, cols):
                t = const.tile([128, cols], FP32, tag=nm, name=f"{nm}_sb")
                nc.sync.dma_start(t, dram_t[:, :])
                return t

            qb_sb = load_small(qb2, "qb", 2)
            kb_sb = load_small(kb2, "kb", 2)
            vb_sb = load_small(vb2, "vb", 2)
            ob_sb = load_small(ob2, "ob", 2)
            mask_sb = load_small(mask01, "mask", KT) if apply_mask else None

            ones_sb = const.tile([128, DH], BF16_DT, tag="ones", name="ones_sb")
            nc.vector.memset(ones_sb, 1.0)
            onesf_sb = const.tile([128, DH], FP32, tag="onesf", name="onesf_sb")
            nc.vector.memset(onesf_sb, 1.0)

            # Warm the PE clock gate while inputs stream in: dependency-free
            # matmuls on the ones tile keep the HAM busy-window active.
            wsc = const.tile([128, 512], BF16_DT, tag="wsc", name="wsc")
            nc.vector.memset(wsc, 0.0)
            wps = ps.tile([128, 512], FP32, tag="sc", bufs=2, name="wps")
            for _ in range(WARMUP_MM):
                nc.tensor.matmul(wps[0:64, :], lhsT=ones_sb[:, 0:64], rhs=wsc)

            # ---- k/q projections (transposed: own-dim on partitions) ----
            # k is stored zero-padded to the full 128-partition contraction:
            # kTp[p][h] has head h's 64 dims in its own partition rows and
            # zeros in the other head's rows, so the QK^T matmul streams the
            # full 128-partition q tile at full SBUF bandwidth.
            qT_sb = [
                const.tile([128, L], BF16_DT, tag=f"qT{p}", name=f"qT_sb{p}")
                for p in range(2)
            ]
            kTp_sb = [
                [
                    const.tile([128, L], BF16_DT, tag=f"kT{p}{h}", name=f"kTp_sb{p}{h}")
                    for h in range(2)
                ]
                for p in range(2)
            ]
            for p in range(2):
                nc.vector.memset(kTp_sb[p][0][64:128, :], 0.0)
                nc.vector.memset(kTp_sb[p][1][0:64, :], 0.0)
            for m in range(2):
                for n in range(NT):
                    nsl = slice(n * 512, (n + 1) * 512)
                    pp = ps.tile([128, 512], FP32, tag="sc", bufs=2, name="projk_ps")
                    for kc in range(KC_D):
                        nc.tensor.matmul(
                            pp,
                            lhsT=wk_sb[kc][:, m * 128 : (m + 1) * 128],
                            rhs=x_sb[kc][:, nsl],
                            start=(kc == 0),
                            stop=(kc == KC_D - 1),
                        )
                    nc.vector.tensor_scalar_add(
                        kTp_sb[m][0][0:64, nsl], pp[0:64, :], kb_sb[0:64, m : m + 1]
                    )
                    nc.vector.tensor_scalar_add(
                        kTp_sb[m][1][64:128, nsl], pp[64:128, :], kb_sb[64:128, m : m + 1]
                    )
            for m in range(2):
                for n in range(NT):
                    nsl = slice(n * 512, (n + 1) * 512)
                    pp = ps.tile([128, 512], FP32, tag="sc", bufs=2, name="projq_ps")
                    for kc in range(KC_D):
                        nc.tensor.matmul(
                            pp,
                            lhsT=wq_sb[kc][:, m * 128 : (m + 1) * 128],
                            rhs=x_sb[kc][:, nsl],
                            start=(kc == 0),
                            stop=(kc == KC_D - 1),
                        )
                    nc.vector.tensor_scalar_add(
                        qT_sb[m][:, nsl], pp, qb_sb[:, m : m + 1]
                    )

            # ---- v projection (untransposed: tokens on partitions; no bias) ----
            v_sb = [
                const.tile([128, OWN], BF16_DT, tag=f"v{t}", name=f"v_sb{t}")
                for t in range(KT)
            ]
            for t in range(KT):
                pv = ps.tile([128, OWN], FP32, tag="sc", bufs=2, name="v_ps")
                for kc in range(KC_D):
                    nc.tensor.matmul(
                        pv,
                        lhsT=x_sb[kc][:, t * 128 : (t + 1) * 128],
                        rhs=wv_sb[kc],
                        start=(kc == 0),
                        stop=(kc == KC_D - 1),
                    )
                nc.vector.tensor_copy(v_sb[t], pv)

            # ---- attention + per-token-tile AllGather / output projection ----
            ctx_sb = [
                const.tile([128, L], BF16_DT, tag=f"ctx{p}", name=f"ctx_sb{p}")
                for p in range(2)
            ]
            n_grp = KT // ACT_GRP
            def oproj(qt):
                # output projection for token tile qt (own 256 columns);
                # emitted one tile behind attention so the tensor engine never
                # queues behind the AllGather latency
                qsl = slice(qt * 512, (qt + 1) * 512)
                cf = []
                for kc in range(KC_D):
                    t = work.tile([128, 512], BF16_DT, tag=f"cf{kc}", name=f"cf{kc}")
                    nc.sync.dma_start(t, ag_out[qt][kc * 128 : (kc + 1) * 128, :])
                    cf.append(t)
                for m in range(2):
                    po = ps.tile([128, 512], FP32, tag="op", bufs=2, name="o_ps")
                    for kc in range(KC_D):
                        nc.tensor.matmul(
                            po,
                            lhsT=wo_sb[kc][:, m * 128 : (m + 1) * 128],
                            rhs=cf[kc],
                            start=(kc == 0),
                            stop=(kc == KC_D - 1),
                        )
                    osb = work.tile([128, 512], FP32, tag="osb", name="osb")
                    nc.vector.tensor_scalar_add(osb, po, ob_sb[:, m : m + 1])
                    nc.sync.dma_start(outT[m * 128 : (m + 1) * 128, qsl], osb)

            ag_out = []
            for qt in range(NT):
                qsl = slice(qt * 512, (qt + 1) * 512)
                for p in range(2):
                    ctx_ps = ps.tile([128, 512], FP32, tag="ctx", bufs=1, name="ctx_ps")
                    sum_ps = ps.tile([128, 512], FP32, tag="sums", bufs=1, name="sum_ps")
                    qfull = qT_sb[p][:, qsl]

                    def av_group(g, pr0, pr1):
                        for j in range(ACT_GRP):
                            kc = g * ACT_GRP + j
                            sl = slice(j * 512, (j + 1) * 512)
                            st = kc == 0
                            sp = kc == KT - 1
                            nc.tensor.matmul(
                                ctx_ps[0:64, :],
                                lhsT=v_sb[kc][:, p * 128 : p * 128 + 64],
                                rhs=pr0[:, sl],
                                start=st,
                                stop=sp,
                            )
                            nc.tensor.matmul(
                                ctx_ps[64:128, :],
                                lhsT=v_sb[kc][:, p * 128 + 64 : p * 128 + 128],
                                rhs=pr1[:, sl],
                                start=st,
                                stop=sp,
                            )
                            # denominators, replicated across 64 partitions so
                            # normalize is elementwise
                            nc.tensor.matmul(
                                sum_ps[0:64, :],
                                lhsT=ones_sb,
                                rhs=pr0[:, sl],
                                start=st,
                                stop=sp,
                            )
                            nc.tensor.matmul(
                                sum_ps[64:128, :],
                                lhsT=ones_sb,
                                rhs=pr1[:, sl],
                                start=st,
                                stop=sp,
                            )

                    prev = None
                    for g in range(n_grp):
                        s0 = ps.tile(
                            [128, 512 * ACT_GRP], FP32, tag="sc", bufs=2, name="s0"
                        )
                        s1 = ps.tile(
                            [128, 512 * ACT_GRP], FP32, tag="sc", bufs=2, name="s1"
                        )
                        for j in range(ACT_GRP):
                            kc = g * ACT_GRP + j
                            nc.tensor.matmul(
                                s0[:, j * 512 : (j + 1) * 512],
                                lhsT=kTp_sb[p][0][:, kc * 128 : (kc + 1) * 128],
                                rhs=qfull,
                            )
                            nc.tensor.matmul(
                                s1[:, j * 512 : (j + 1) * 512],
                                lhsT=kTp_sb[p][1][:, kc * 128 : (kc + 1) * 128],
                                rhs=qfull,
                            )
                        pr0 = work.tile(
                            [128, 512 * ACT_GRP], BF16_DT, tag="pr0", name="pr0"
                        )
                        pr1 = work.tile(
                            [128, 512 * ACT_GRP], BF16_DT, tag="pr1", name="pr1"
                        )
                        if apply_mask:
                            e0 = work.tile(
                                [128, 512 * ACT_GRP], FP32, tag="e0", name="e0"
                            )
                            e1 = work.tile(
                                [128, 512 * ACT_GRP], FP32, tag="e1", name="e1"
                            )
                            nc.scalar.activation(e0, s0, Exp)
                            nc.scalar.activation(e1, s1, Exp)
                            for j in range(ACT_GRP):
                                kc = g * ACT_GRP + j
                                sl = slice(j * 512, (j + 1) * 512)
                                nc.vector.tensor_scalar_mul(
                                    pr0[:, sl], e0[:, sl], mask_sb[:, kc : kc + 1]
                                )
                                nc.vector.tensor_scalar_mul(
                                    pr1[:, sl], e1[:, sl], mask_sb[:, kc : kc + 1]
                                )
                        else:
                            nc.scalar.activation(pr0, s0, Exp)
                            nc.scalar.activation(pr1, s1, Exp)
                        # AV runs one exp-group behind so the tensor engine
                        # never stalls waiting on the current group's exp
                        if prev is not None:
                            av_group(g - 1, *prev)
                        prev = (pr0, pr1)
                    av_group(n_grp - 1, *prev)

                    # normalize: full-width fast reciprocal (base-0, the only
                    # base the custom-DVE op supports), then elementwise
                    recip = work.tile([128, 512], FP32, tag="recip", name="recip")
                    nc.vector.reciprocal_approx_fast(recip, sum_ps)
                    tmp = work.tile([128, 512], FP32, tag="tmp", name="tmp")
                    nc.vector.tensor_mul(tmp, ctx_ps, recip)
                    nc.vector.tensor_scalar_add(
                        ctx_sb[p][:, qsl], tmp, vb_sb[:, p : p + 1]
                    )

                # AllGather this token tile's context within the batch group
                ag_in = dram.tile([OWN, 512], BF16_DT, tag=f"agi{qt}", name=f"agi{qt}")
                ago = dram.tile([D, 512], BF16_DT, tag=f"ago{qt}", name=f"ago{qt}")
                ag_out.append(ago)
                nc.sync.dma_start(ag_in[0:128, :], ctx_sb[0][:, qsl])
                nc.sync.dma_start(ag_in[128:256, :], ctx_sb[1][:, qsl])
                nc.gpsimd.collective_compute(
                    "AllGather",
                    mybir.AluOpType.bypass,
                    replica_groups=[[0, 1, 2, 3], [4, 5, 6, 7]],
                    ins=[ag_in.opt()],
                    outs=[ago.opt()],
                )
                if qt > 0:
                    oproj(qt - 1)
            oproj(NT - 1)

    nc.finalize()
    return nc


@functools.lru_cache(maxsize=2)
def _built(apply_mask: bool):
    return _build_nc(apply_mask)


def kernel(input, mask, q_w, q_b, k_w, k_b, v_w, v_b, o_w, o_b):
    global LAST_EXEC_NS, LAST_RESULTS
    input = np.asarray(input, dtype=np.float32)
    mask = np.asarray(mask)
    apply_mask = not bool(np.all(mask != 0))
    nc = _built(apply_mask)

    qw = (np.asarray(q_w, np.float32) / 8.0).astype(BF16)
    kw = np.asarray(k_w, np.float32).astype(BF16)
    vw = np.asarray(v_w, np.float32).astype(BF16)
    ow = np.asarray(o_w, np.float32).astype(BF16)
    qb = np.asarray(q_b, np.float32) / 8.0
    kb = np.asarray(k_b, np.float32)
    vb = np.asarray(v_b, np.float32)
    ob = np.asarray(o_b, np.float32)

    in_maps = []
    for c in range(N_CORES):
        b, r = divmod(c, 4)
        own = slice(OWN * r, OWN * (r + 1))
        m01 = (mask[b] != 0).astype(np.float32)
        in_maps.append(
            {
                "xT": np.ascontiguousarray(input[b].T.astype(BF16)),
                "wq": np.ascontiguousarray(qw[own, :].T),
                "wk": np.ascontiguousarray(kw[own, :].T),
                "wv": np.ascontiguousarray(vw[own, :].T),
                "wo": np.ascontiguousarray(ow[own, :].T),
                "qb2": np.ascontiguousarray(qb[own].reshape(2, 128).T),
                "kb2": np.ascontiguousarray(kb[own].reshape(2, 128).T),
                "vb2": np.ascontiguousarray(vb[own].reshape(2, 128).T),
                "ob2": np.ascontiguousarray(ob[own].reshape(2, 128).T),
                "mask01": np.ascontiguousarray(m01.reshape(KT, 128).T),
            }
        )

    trace = os.environ.get("KERNEL_TRACE", "0") == "1"
    res = run_bass_kernel_spmd(
        nc,
        in_maps,
        core_ids=list(range(N_CORES)),
        trace=trace,
        trace_cores=list(range(N_CORES)) if trace else None,
        stitch_traces=False,
    )
    LAST_EXEC_NS = res.exec_time_ns
    LAST_RESULTS = res

    out = np.empty((BS, L, D), dtype=np.float32)
    for c in range(N_CORES):
        b, r = divmod(c, 4)
        out[b, :, OWN * r : OWN * (r + 1)] = res.results[c]["outT"].T
    return out


# revision 27
# speedup vs baseline: 1.2386x; 1.0607x over previous
"""Multi-head attention (16 heads, d_model=1024, bs=2, qlen=2048) on 8 trn2 cores.

Sharding: core c -> batch b = c//4, head-group r = c%4 (heads 4r..4r+3, i.e.
dims 256r..256r+256 of the head axis).  Each core projects q/k/v only for its
own 4 heads (Megatron column split), runs scores/softmax/AV for those heads,
then an AllGather of the per-core context slice within each batch group of 4
cores provides the full 1024-dim context for the row-split output projection
(each core computes its own 256 output columns; no reduction needed).  The
AllGather + output projection are split per 512-token tile and overlap the
attention of later tiles.

Numerics: bf16 matmul operands, fp32 PSUM accumulation, softmax in fp32 on the
scalar engine.  Scores are computed transposed (k on partitions) so the mask
is a per-partition multiply and the softmax denominator comes from an M=1
ones-matmul; the reciprocal is computed on a single partition row and
broadcast across partitions with a K=1 ones-matmul (elementwise normalize, no
cross-partition vector ops).  1/sqrt(d) and q_b are folded into q_w/q_b on the
host; v_b is deferred past the softmax (rows of P/sum sum to 1) and added to
the normalized context.
"""

import functools
import os
import sys

import numpy as np

for _p in ("/opt/trn_rl_repo", "/root/.axon_site/_ro/trn_rl_repo"):
    if os.path.isdir(_p) and _p not in sys.path:
        sys.path.append(_p)

import ml_dtypes

from concourse import bacc, bass, mybir, tile
from concourse.bass_utils import run_bass_kernel_spmd

BF16 = ml_dtypes.bfloat16
FP32 = mybir.dt.float32
BF16_DT = mybir.dt.bfloat16

N_CORES = 8
BS = 2
L = 2048  # sequence length
D = 1024  # model dim
DH = 64  # head dim
OWN = 256  # head dims per core (4 heads)
KC_D = 8  # 1024 / 128 contraction chunks for projections
NT = 4  # 2048 / 512 token tiles
KT = 16  # 2048 / 128 key-token chunks
ACT_GRP = 2  # k-chunks per exp() activation op
WARMUP_MM = 28  # dummy matmuls to warm the PE clock gate during input DMA

LAST_EXEC_NS = None
LAST_RESULTS = None


def _build_nc(apply_mask: bool):
    nc = bacc.Bacc(None, num_devices=N_CORES)

    xT = nc.dram_tensor("xT", [D, L], BF16_DT, kind="ExternalInput")
    wq = nc.dram_tensor("wq", [D, OWN], BF16_DT, kind="ExternalInput")
    wk = nc.dram_tensor("wk", [D, OWN], BF16_DT, kind="ExternalInput")
    wv = nc.dram_tensor("wv", [D, OWN], BF16_DT, kind="ExternalInput")
    wo = nc.dram_tensor("wo", [D, OWN], BF16_DT, kind="ExternalInput")
    qb2 = nc.dram_tensor("qb2", [128, 2], FP32, kind="ExternalInput")
    kb2 = nc.dram_tensor("kb2", [128, 2], FP32, kind="ExternalInput")
    vb2 = nc.dram_tensor("vb2", [128, 2], FP32, kind="ExternalInput")
    ob2 = nc.dram_tensor("ob2", [128, 2], FP32, kind="ExternalInput")
    mask01 = nc.dram_tensor("mask01", [128, KT], FP32, kind="ExternalInput")
    outT = nc.dram_tensor("outT", [OWN, L], FP32, kind="ExternalOutput")

    Exp = mybir.ActivationFunctionType.Exp

    with tile.TileContext(nc) as tc:
        with (
            tc.tile_pool(name="const", bufs=1) as const,
            tc.tile_pool(name="work", bufs=2) as work,
            tc.tile_pool(name="ps", bufs=1, space="PSUM") as ps,
            tc.tile_pool(name="dram", bufs=1, space="DRAM") as dram,
        ):
            # ---- stage inputs into SBUF ----
            def load_w(dram_t, nm):
                tiles = []
                for i in range(KC_D):
                    t = const.tile([128, OWN], BF16_DT, tag=f"{nm}{i}", name=f"{nm}_sb{i}")
                    nc.sync.dma_start(t, dram_t[i * 128 : (i + 1) * 128, :])
                    tiles.append(t)
                return tiles

            wk_sb = load_w(wk, "wk")
            # x split per (chunk, n-tile), n-major, so the first projection
            # group can start after 1/4 of the input has landed
            x_sb = [
                const.tile([128, L], BF16_DT, tag=f"x{i}", name=f"x_sb{i}")
                for i in range(KC_D)
            ]
            for n in range(NT):
                for i in range(KC_D):
                    nc.sync.dma_start(
                        x_sb[i][:, n * 512 : (n + 1) * 512],
                        xT[i * 128 : (i + 1) * 128, n * 512 : (n + 1) * 512],
                    )
            wq_sb = load_w(wq, "wq")
            wv_sb = load_w(wv, "wv")
            wo_sb = load_w(wo, "wo")

            def load_small(dram_t, nm, cols):
                t = const.tile([128, cols], FP32, tag=nm, name=f"{nm}_sb")
                nc.sync.dma_start(t, dram_t[:, :])
                return t

            qb_sb = load_small(qb2, "qb", 2)
            kb_sb = load_small(kb2, "kb", 2)
            vb_sb = load_small(vb2, "vb", 2)
            ob_sb = load_small(ob2, "ob", 2)
            mask_sb = load_small(mask01, "mask", KT) if apply_mask else None

            ones_sb = const.tile([128, DH], BF16_DT, tag="ones", name="ones_sb")
            nc.vector.memset(ones_sb, 1.0)
            onesf_sb = const.tile([128, DH], FP32, tag="onesf", name="onesf_sb")
            nc.vector.memset(onesf_sb, 1.0)

            # Warm the PE clock gate while inputs stream in: dependency-free
            # matmuls on the ones tile keep the HAM busy-window active.
            wsc = const.tile([128, 512], BF16_DT, tag="wsc", name="wsc")
            nc.vector.memset(wsc, 0.0)
            wps = ps.tile([128, 512], FP32, tag="sc", bufs=2, name="wps")
            for _ in range(WARMUP_MM):
                nc.tensor.matmul(wps[0:64, :], lhsT=ones_sb[:, 0:64], rhs=wsc)

            # ---- k/q projections (transposed: own-dim on partitions) ----
            # k is stored zero-padded to the full 128-partition contraction:
            # kTp[p][h] has head h's 64 dims in its own partition rows and
            # zeros in the other head's rows, so the QK^T matmul streams the
            # full 128-partition q tile at full SBUF bandwidth.
            qT_sb = [
                const.tile([128, L], BF16_DT, tag=f"qT{p}", name=f"qT_sb{p}")
                for p in range(2)
            ]
            kTp_sb = [
                [
                    const.tile([128, L], BF16_DT, tag=f"kT{p}{h}", name=f"kTp_sb{p}{h}")
                    for h in range(2)
                ]
                for p in range(2)
            ]
            for p in range(2):
                nc.vector.memset(kTp_sb[p][0][64:128, :], 0.0)
                nc.vector.memset(kTp_sb[p][1][0:64, :], 0.0)
            for m in range(2):
                for n in range(NT):
                    nsl = slice(n * 512, (n + 1) * 512)
                    pp = ps.tile([128, 512], FP32, tag="sc", bufs=2, name="projk_ps")
                    for kc in range(KC_D):
                        nc.tensor.matmul(
                            pp,
                            lhsT=wk_sb[kc][:, m * 128 : (m + 1) * 128],
                            rhs=x_sb[kc][:, nsl],
                            start=(kc == 0),
                            stop=(kc == KC_D - 1),
                        )
                    nc.vector.tensor_scalar_add(
                        kTp_sb[m][0][0:64, nsl], pp[0:64, :], kb_sb[0:64, m : m + 1]
                    )
                    nc.vector.tensor_scalar_add(
                        kTp_sb[m][1][64:128, nsl], pp[64:128, :], kb_sb[64:128, m : m + 1]
                    )
            for m in range(2):
                for n in range(NT):
                    nsl = slice(n * 512, (n + 1) * 512)
                    pp = ps.tile([128, 512], FP32, tag="sc", bufs=2, name="projq_ps")
                    for kc in range(KC_D):
                        nc.tensor.matmul(
                            pp,
                            lhsT=wq_sb[kc][:, m * 128 : (m + 1) * 128],
                            rhs=x_sb[kc][:, nsl],
                            start=(kc == 0),
                            stop=(kc == KC_D - 1),
                        )
                    nc.vector.tensor_scalar_add(
                        qT_sb[m][:, nsl], pp, qb_sb[:, m : m + 1]
                    )

            # ---- v projection (untransposed: tokens on partitions; no bias) ----
            # Stored interleaved with ones columns so a single M=128 matmul
            # per head produces both context rows and softmax-denominator
            # rows: per pair block of 256 cols: [v_h0 | ones | ones | v_h1].
            v_sb = [
                const.tile([128, 512], BF16_DT, tag=f"v{t}", name=f"v_sb{t}")
                for t in range(KT)
            ]
            for t in range(KT):
                nc.vector.memset(v_sb[t][:, 64:192], 1.0)
                nc.vector.memset(v_sb[t][:, 320:448], 1.0)
                pv = ps.tile([128, OWN], FP32, tag="sc", bufs=2, name="v_ps")
                for kc in range(KC_D):
                    nc.tensor.matmul(
                        pv,
                        lhsT=x_sb[kc][:, t * 128 : (t + 1) * 128],
                        rhs=wv_sb[kc],
                        start=(kc == 0),
                        stop=(kc == KC_D - 1),
                    )
                nc.vector.tensor_copy(v_sb[t][:, 0:64], pv[:, 0:64])
                nc.vector.tensor_copy(v_sb[t][:, 192:256], pv[:, 64:128])
                nc.vector.tensor_copy(v_sb[t][:, 256:320], pv[:, 128:192])
                nc.vector.tensor_copy(v_sb[t][:, 448:512], pv[:, 192:256])

            # ---- attention + per-token-tile AllGather / output projection ----
            ctx_sb = [
                const.tile([128, L], BF16_DT, tag=f"ctx{p}", name=f"ctx_sb{p}")
                for p in range(2)
            ]
            n_grp = KT // ACT_GRP
            def oproj(qt):
                # output projection for token tile qt (own 256 columns);
                # emitted one tile behind attention so the tensor engine never
                # queues behind the AllGather latency.  AllGather output row
                # r*128+i of pair p holds global dim 256r+128p+i, i.e. wo
                # chunk 2r+p.
                qsl = slice(qt * 512, (qt + 1) * 512)
                cf = []
                for r in range(4):
                    for p in range(2):
                        t = work.tile(
                            [128, 512], BF16_DT, tag=f"cf{r}{p}", name=f"cf{r}{p}"
                        )
                        nc.sync.dma_start(
                            t, ag_out[qt][p][r * 128 : (r + 1) * 128, :]
                        )
                        cf.append((2 * r + p, t))
                for m in range(2):
                    po = ps.tile([128, 512], FP32, tag="op", bufs=2, name="o_ps")
                    for i, (kc, t) in enumerate(cf):
                        nc.tensor.matmul(
                            po,
                            lhsT=wo_sb[kc][:, m * 128 : (m + 1) * 128],
                            rhs=t,
                            start=(i == 0),
                            stop=(i == KC_D - 1),
                        )
                    osb = work.tile([128, 512], FP32, tag="osb", name="osb")
                    nc.vector.tensor_scalar_add(osb, po, ob_sb[:, m : m + 1])
                    nc.sync.dma_start(outT[m * 128 : (m + 1) * 128, qsl], osb)

            ag_out = [[] for _ in range(NT)]
            for qt in range(NT):
                qsl = slice(qt * 512, (qt + 1) * 512)
                for p in range(2):
                    # cs0: rows 0-63 ctx_h0, rows 64-127 denominators (x64)
                    # cs1: rows 0-63 denominators (x64), rows 64-127 ctx_h1
                    cs0 = ps.tile([128, 512], FP32, tag="ctx", bufs=1, name="cs0")
                    cs1 = ps.tile([128, 512], FP32, tag="sums", bufs=1, name="cs1")
                    qfull = qT_sb[p][:, qsl]

                    def av_group(g, pr0, pr1):
                        for j in range(ACT_GRP):
                            kc = g * ACT_GRP + j
                            sl = slice(j * 512, (j + 1) * 512)
                            st = kc == 0
                            sp = kc == KT - 1
                            nc.tensor.matmul(
                                cs0,
                                lhsT=v_sb[kc][:, p * 256 : p * 256 + 128],
                                rhs=pr0[:, sl],
                                start=st,
                                stop=sp,
                            )
                            nc.tensor.matmul(
                                cs1,
                                lhsT=v_sb[kc][:, p * 256 + 128 : p * 256 + 256],
                                rhs=pr1[:, sl],
                                start=st,
                                stop=sp,
                            )

                    prev = None
                    for g in range(n_grp):
                        s0 = ps.tile(
                            [128, 512 * ACT_GRP], FP32, tag="sc", bufs=2, name="s0"
                        )
                        s1 = ps.tile(
                            [128, 512 * ACT_GRP], FP32, tag="sc", bufs=2, name="s1"
                        )
                        for j in range(ACT_GRP):
                            kc = g * ACT_GRP + j
                            nc.tensor.matmul(
                                s0[:, j * 512 : (j + 1) * 512],
                                lhsT=kTp_sb[p][0][:, kc * 128 : (kc + 1) * 128],
                                rhs=qfull,
                            )
                            nc.tensor.matmul(
                                s1[:, j * 512 : (j + 1) * 512],
                                lhsT=kTp_sb[p][1][:, kc * 128 : (kc + 1) * 128],
                                rhs=qfull,
                            )
                        pr0 = work.tile(
                            [128, 512 * ACT_GRP], BF16_DT, tag="pr0", name="pr0"
                        )
                        pr1 = work.tile(
                            [128, 512 * ACT_GRP], BF16_DT, tag="pr1", name="pr1"
                        )
                        if apply_mask:
                            e0 = work.tile(
                                [128, 512 * ACT_GRP], FP32, tag="e0", name="e0"
                            )
                            e1 = work.tile(
                                [128, 512 * ACT_GRP], FP32, tag="e1", name="e1"
                            )
                            nc.scalar.activation(e0, s0, Exp)
                            nc.scalar.activation(e1, s1, Exp)
                            for j in range(ACT_GRP):
                                kc = g * ACT_GRP + j
                                sl = slice(j * 512, (j + 1) * 512)
                                nc.vector.tensor_scalar_mul(
                                    pr0[:, sl], e0[:, sl], mask_sb[:, kc : kc + 1]
                                )
                                nc.vector.tensor_scalar_mul(
                                    pr1[:, sl], e1[:, sl], mask_sb[:, kc : kc + 1]
                                )
                        else:
                            nc.scalar.activation(pr0, s0, Exp)
                            nc.scalar.activation(pr1, s1, Exp)
                        # AV runs one exp-group behind so the tensor engine
                        # never stalls waiting on the current group's exp
                        if prev is not None:
                            av_group(g - 1, *prev)
                        prev = (pr0, pr1)
                    av_group(n_grp - 1, *prev)

                    # normalize: copy one denominator row per head to SBUF,
                    # broadcast across partitions with K=1 ones-matmuls into a
                    # base-0 bank, take the fast reciprocal there (the
                    # custom-DVE reciprocal only supports base 0), multiply
                    srow = work.tile([128, 512], FP32, tag="srow", name="srow")
                    nc.vector.tensor_copy(srow[64:65, :], cs0[64:65, :])
                    nc.vector.tensor_copy(srow[0:1, :], cs1[0:1, :])
                    rb = ps.tile([128, 512], FP32, tag="op", bufs=2, name="rb")
                    nc.tensor.matmul(
                        rb[0:64, :], lhsT=onesf_sb[64:65, 0:64], rhs=srow[64:65, :]
                    )
                    nc.tensor.matmul(
                        rb[64:128, :], lhsT=onesf_sb[0:1, 0:64], rhs=srow[0:1, :]
                    )
                    recip = work.tile([128, 512], FP32, tag="recip", name="recip")
                    nc.vector.reciprocal_approx_fast(recip, rb)
                    tmp = work.tile([128, 512], FP32, tag="tmp", name="tmp")
                    nc.vector.tensor_mul(tmp[0:64, :], cs0[0:64, :], recip[0:64, :])
                    nc.vector.tensor_mul(
                        tmp[64:128, :], cs1[64:128, :], recip[64:128, :]
                    )
                    nc.vector.tensor_scalar_add(
                        ctx_sb[p][:, qsl], tmp, vb_sb[:, p : p + 1]
                    )

                    # AllGather this pair's context slice within the batch
                    # group as soon as it is ready
                    ag_in = dram.tile(
                        [128, 512], BF16_DT, tag=f"agi{qt}{p}", name=f"agi{qt}{p}"
                    )
                    ago = dram.tile(
                        [512, 512], BF16_DT, tag=f"ago{qt}{p}", name=f"ago{qt}{p}"
                    )
                    ag_out[qt].append(ago)
                    nc.sync.dma_start(ag_in[:, :], ctx_sb[p][:, qsl])
                    nc.gpsimd.collective_compute(
                        "AllGather",
                        mybir.AluOpType.bypass,
                        replica_groups=[[0, 1, 2, 3], [4, 5, 6, 7]],
                        ins=[ag_in.opt()],
                        outs=[ago.opt()],
                    )
                if qt > 0:
                    oproj(qt - 1)
            oproj(NT - 1)

    nc.finalize()
    return nc


@functools.lru_cache(maxsize=2)
def _built(apply_mask: bool):
    return _build_nc(apply_mask)


def kernel(input, mask, q_w, q_b, k_w, k_b, v_w, v_b, o_w, o_b):
    global LAST_EXEC_NS, LAST_RESULTS
    input = np.asarray(input, dtype=np.float32)
    mask = np.asarray(mask)
    apply_mask = not bool(np.all(mask != 0))
    nc = _built(apply_mask)

    qw = (np.asarray(q_w, np.float32) / 8.0).astype(BF16)
    kw = np.asarray(k_w, np.float32).astype(BF16)
    vw = np.asarray(v_w, np.float32).astype(BF16)
    ow = np.asarray(o_w, np.float32).astype(BF16)
    qb = np.asarray(q_b, np.float32) / 8.0
    kb = np.asarray(k_b, np.float32)
    vb = np.asarray(v_b, np.float32)
    ob = np.asarray(o_b, np.float32)

    in_maps = []
    for c in range(N_CORES):
        b, r = divmod(c, 4)
        own = slice(OWN * r, OWN * (r + 1))
        m01 = (mask[b] != 0).astype(np.float32)
        in_maps.append(
            {
                "xT": np.ascontiguousarray(input[b].T.astype(BF16)),
                "wq": np.ascontiguousarray(qw[own, :].T),
                "wk": np.ascontiguousarray(kw[own, :].T),
                "wv": np.ascontiguousarray(vw[own, :].T),
                "wo": np.ascontiguousarray(ow[own, :].T),
                "qb2": np.ascontiguousarray(qb[own].reshape(2, 128).T),
                "kb2": np.ascontiguousarray(kb[own].reshape(2, 128).T),
                "vb2": np.ascontiguousarray(vb[own].reshape(2, 128).T),
                "ob2": np.ascontiguousarray(ob[own].reshape(2, 128).T),
                "mask01": np.ascontiguousarray(m01.reshape(KT, 128).T),
            }
        )

    trace = os.environ.get("KERNEL_TRACE", "0") == "1"
    res = run_bass_kernel_spmd(
        nc,
        in_maps,
        core_ids=list(range(N_CORES)),
        trace=trace,
        trace_cores=list(range(N_CORES)) if trace else None,
        stitch_traces=False,
    )
    LAST_EXEC_NS = res.exec_time_ns
    LAST_RESULTS = res

    out = np.empty((BS, L, D), dtype=np.float32)
    for c in range(N_CORES):
        b, r = divmod(c, 4)
        out[b, :, OWN * r : OWN * (r + 1)] = res.results[c]["outT"].T
    return out
